# revision 38
# baseline (speedup 1.0000x reference)
"""Trainium2 Bass kernel for a BFP-quantized ResNet BasicBlock (inference).

Computes, per image (NCHW, C=128, H=W=56):
    out = relu( bn2( conv3x3( q( relu(bn1( conv3x3(q(x), q(w1)) )) ), q(w2)) ) + x )
where q() is HBFP block-floating-point quantization: blocks of 64 contiguous
values (flat row-major) share a power-of-2 scale 2^(floor(log2(max|x|))-7),
mantissas RNE-rounded to 8 signed bits and clamped to +-127.

Exact-semantics quant chain, engine-balanced:
  V: absmax-reduce -> exponent-field block scales -> normalize mult (f32)
  G: RNE round (magic-constant dual op, f32 -> bf16; integer mantissas are
     bf16-exact) -> clip to +-127 in bf16 (2x rate; round-then-clip is
     exactly the reference's clip(round(t))) -> scale-back mult in bf16
     (power-of-2 scales are bf16-exact).

Pipeline: quant2 is emitted in 3 row-aligned splits, each followed by its
own pad DMA, so conv2 chunk groups {0,1},{2,3},{4,5,6} start as soon as
their input rows exist.  conv1 chunks of image n+1 are interleaved between
conv2 groups of image n so the PE queue never starves.  Residual add runs
on GpSimd, final relu on Scalar, both per conv2-group.

Sharding: data-parallel over batch N=64 -> 8 images per NeuronCore, weights
and BN constants replicated. All 8 cores run the same NEFF (SPMD).
"""

import os

os.environ.setdefault("MYCRO_LOCAL_CACHE", "1")

from contextlib import ExitStack
from functools import lru_cache

import numpy as np

import concourse.bass as bass
import concourse.tile as tile
from concourse import bacc, mybir
from concourse.bass_utils import run_bass_kernel_spmd
from concourse.masks import make_identity

P = 128
H = W = 56
HWF = H * W            # 3136 flat pixels per channel
NBX = HWF // 64        # 49 BFP blocks per channel image
WLEN = 128 * 9         # 1152 flat weight row per output channel
NBW = WLEN // 64       # 18 BFP blocks per weight row
PITCH = W + 2          # 58 padded row pitch
PADLEN = PITCH * PITCH + 2  # 3366: [1 pre-pad][58x58 padded image][1 post-pad]
NCHUNK = 7             # 8-row output chunks per image
CHF = 8 * PITCH        # 464 matmul free dim per chunk
CROUND = 12582912.0    # 1.5 * 2**23  (RNE magic constant)
EXPMASK = 0x7F800000
BIAS7 = 7 << 23
C254 = 254 << 23       # rscale bits = C254 - scale_bits
EGUARD = 50 << 23      # exponent field of 1e-23 (reference's zero-guard)
BN_EPS = 1e-5

F32 = mybir.dt.float32
F16 = mybir.dt.float16
BF16 = mybir.dt.bfloat16
I32 = mybir.dt.int32
ALU = mybir.AluOpType
ACTF = mybir.ActivationFunctionType
AX = mybir.AxisListType

N_CORES = 8
NIMG = 8  # images per core

# quant2 splits (BFP blocks) and the conv2 chunk groups they unlock.
# split j covers mid cols [b0*64,(b0+bn)*64) = image rows [b0*64/56, ...):
#   s0: blocks 0..27  -> rows  0..31  -> conv2 chunks 0..2 (need in-rows <=24)
#   s1: blocks 28..48 -> rows 32..55  -> conv2 chunks 3..6
SPLITS = [(0, 28), (28, 21)]
GROUPS = [(0, 3), (3, 7)]


def _emit_quant_split(nc, small, tmp_pool, src_ap, dst_ap, bn, tag,
                      inplace=False):
    """Exactly BFP-quantize src_ap (f32 [P, bn*64]) into dst_ap (bf16, same
    shape).  Blocks of 64 along the free dim share a scale.

    V: reduce + block-scale bits + normalize mult.
    G: RNE round (f32->bf16) + clip +-127 (bf16) + scale-back (bf16).
    If inplace, the normalize mult overwrites src_ap (src dead afterwards).
    """
    src3 = src_ap.rearrange("p (b e) -> p b e", e=64)
    dst3 = dst_ap.rearrange("p (b e) -> p b e", e=64)

    bm = small.tile([P, bn], F32, tag=f"bm_{tag}")
    sbx = small.tile([P, bn], I32, tag=f"sbx_{tag}")
    rb = small.tile([P, bn], I32, tag=f"rb_{tag}")
    sbits = small.tile([P, bn], I32, tag=f"sbits_{tag}")
    if inplace:
        t3 = src3
    else:
        t = tmp_pool.tile([P, bn * 64], F32, tag=f"qt_{tag}", bufs=1)
        t3 = t[:].rearrange("p (b e) -> p b e", e=64)

    nc.vector.tensor_reduce(
        out=bm[:], in_=src3, axis=AX.X,
        op=ALU.max, apply_absolute_value=True,
    )
    # scale bits = max(expfield, expfield(1e-23)) - (7<<23) -> scale = 2^(e-7)
    nc.vector.tensor_scalar(sbx[:], bm[:].bitcast(I32), EXPMASK, None,
                            ALU.bitwise_and)
    nc.vector.tensor_scalar(sbits[:], sbx[:], EGUARD, BIAS7,
                            ALU.max, ALU.subtract)
    # rscale bits = (254<<23) - scale_bits -> rscale = 2^(7-e)
    nc.vector.tensor_scalar(rb[:], sbits[:], C254, -1,
                            ALU.subtract, ALU.mult)
    # normalize: t = src * 2^(7-e), |t| < 256
    rsc = rb[:].bitcast(F32)[:, :, None].to_broadcast((P, bn, 64))
    nc.vector.tensor_tensor(t3, src3, rsc, ALU.mult)
    # RNE round to integer (f32 magic add/sub), exact bf16 downcast.
    # Whole chain on Vector: concurrent V+G big ops slow each other down
    # (SBUF contention), so a hot V with a mostly-idle G wins.
    nc.vector.tensor_scalar(dst3, t3, CROUND, CROUND, ALU.add, ALU.subtract)
    # clip(round(t), -127, 127): integers, exact in bf16
    nc.vector.tensor_scalar(dst3, dst3, 127.0, -127.0, ALU.min, ALU.max)
    # scale back: mantissa * 2^(e-7) (f32 scale broadcast, bf16 out; the
    # product is exact since scales are powers of two)
    scb = sbits[:].bitcast(F32)[:, :, None].to_broadcast((P, bn, 64))
    nc.vector.tensor_tensor(dst3, dst3, scb, ALU.mult)


def _interior_rows(pad_tile, r0, r1):
    """[P, r1-r0, 56] strided view (pitch 58) of padded rows [r0, r1)."""
    base = 1 + PITCH + 1 + r0 * PITCH
    v = pad_tile[:, base : base + (r1 - r0) * PITCH]
    return v.rearrange("p (h w) -> p h w", w=PITCH)[:, :, :W]


def _emit_conv_chunks(nc, psum_pool, wk, src_pad, c0, c1, evict):
    """3x3 conv chunks [c0,c1) via 9 accumulated matmuls each; evict(c, ps)."""
    for c in range(c0, c1):
        h0 = c * 8
        ps = psum_pool.tile([P, CHF], F32, tag="pschunk")
        for k in range(9):
            kh, kw = divmod(k, 3)
            s = (h0 + kh) * PITCH + kw
            nc.tensor.matmul(
                ps[:], wk[k][:], src_pad[:, s : s + CHF],
                start=(k == 0), stop=(k == 8),
            )
        evict(c, ps)


def build_nc(nimg=NIMG):
    nc = bacc.Bacc("TRN2", target_bir_lowering=False, debug=False,
                   enable_asserts=False)

    x_d = nc.dram_tensor("x", [nimg, P, H, W], F32, kind="ExternalInput").ap()
    w1_d = nc.dram_tensor("w1", [P, P, 3, 3], F32, kind="ExternalInput").ap()
    w2_d = nc.dram_tensor("w2", [P, P, 3, 3], F32, kind="ExternalInput").ap()
    bn_d = {
        name: nc.dram_tensor(name, [P], F32, kind="ExternalInput").ap()
        for name in ("gamma1", "beta1", "mean1", "var1",
                     "gamma2", "beta2", "mean2", "var2")
    }
    out_d = nc.dram_tensor("out", [nimg, P, H, W], F32, kind="ExternalOutput").ap()

    with tile.TileContext(nc) as tc, ExitStack() as ctx:
        const = ctx.enter_context(tc.tile_pool(name="const", bufs=1))
        small = ctx.enter_context(tc.tile_pool(name="small", bufs=6))
        tmp = ctx.enter_context(tc.tile_pool(name="tmp", bufs=3))
        pads = ctx.enter_context(tc.tile_pool(name="pads", bufs=1))
        xraw_p = ctx.enter_context(tc.tile_pool(name="xraw", bufs=5))
        u_p = ctx.enter_context(tc.tile_pool(name="u", bufs=2))
        mid_p = ctx.enter_context(tc.tile_pool(name="mid", bufs=2))
        t2_p = ctx.enter_context(tc.tile_pool(name="t2", bufs=2))

        xraws = [None] * nimg
        mids = [None] * nimg
        t2s = [None] * nimg

        xq_pads = [pads.tile([P, PADLEN], BF16, tag=f"xqp{i}", name=f"xqp{i}")
                   for i in range(3)]
        mq_pads = [pads.tile([P, PADLEN], BF16, tag=f"mqp{i}", name=f"mqp{i}")
                   for i in range(2)]

        # ---- kick off all input DMAs first so they overlap setup compute ----
        def load(n):
            xraw = xraw_p.tile([P, HWF], F32, tag="xraw", name=f"xraw{n}")
            xraws[n] = xraw
            nc.sync.dma_start(xraw[:], x_d[n].rearrange("c h w -> c (h w)"))

        def quant1(n, splits=((0, NBX),)):
            u = u_p.tile([P, HWF], BF16, tag="u", name=f"u{n}")
            for b0, bn in splits:
                src = xraws[n][:, b0 * 64 : (b0 + bn) * 64]
                dst = u[:, b0 * 64 : (b0 + bn) * 64]
                _emit_quant_split(nc, small, tmp, src, dst, bn, f"q1s{b0}")
                r0, r1 = (b0 * 64) // W, ((b0 + bn) * 64) // W
                nc.scalar.dma_start(
                    _interior_rows(xq_pads[n % 3], r0, r1),
                    u[:, r0 * W : r1 * W].rearrange("p (h w) -> p h w", w=W))

        with tc.tile_pool(name="setup", bufs=1) as setup, \
             tc.tile_pool(name="psum_setup", bufs=2, space="PSUM") as psum_setup:
            wraws = []
            for wi, w_d in enumerate((w1_d, w2_d)):
                wraw = setup.tile([P, WLEN], F32, tag=f"wraw{wi}")
                nc.sync.dma_start(wraw[:], w_d.rearrange("o i kh kw -> o (i kh kw)"))
                wraws.append(wraw)
            bnc = {}
            for name in ("gamma1", "beta1", "mean1", "var1",
                         "gamma2", "beta2", "mean2", "var2"):
                t = setup.tile([P, 1], F32, tag=f"bn_{name}")
                nc.sync.dma_start(t[:], bn_d[name][:, None])
                bnc[name] = t
            load(0)
            load(1)

            # ---- weights quant + transpose, BN constants ----
            ident = const.tile([P, P], BF16, tag="ident")
            make_identity(nc, ident[:])
            zero_b = const.tile([P, 1], F32, tag="zero_b")
            nc.vector.memset(zero_b[:], 0.0)
            eps_b = const.tile([P, 1], F32, tag="eps_b")
            nc.vector.memset(eps_b[:], BN_EPS)

            def memset_borders(pad):
                # zero only the halo: top row (+prepad), bottom row
                # (+postpad), and the shared left/right pad columns; the
                # interior is overwritten by the quant pad DMA every image.
                nc.vector.memset(pad[:, 0:60], 0.0)
                cols = pad[:, PITCH : PITCH + 57 * PITCH]
                cols = cols.rearrange("p (k t) -> p k t", t=PITCH)[:, :, 0:2]
                nc.vector.memset(cols, 0.0)
                nc.vector.memset(pad[:, 1 + 57 * PITCH :], 0.0)

            for t in (*xq_pads, *mq_pads):
                memset_borders(t)

            wks = []
            for wi, wraw in enumerate(wraws):
                wq = setup.tile([P, WLEN], BF16, tag=f"wq{wi}")
                _emit_quant_split(nc, setup, setup, wraw[:], wq[:], NBW,
                                  f"w{wi}", inplace=True)
                # per-offset lhsT tiles: w[k][i, o] = wq[o, i*9+k]
                wq_v = wq[:].rearrange("p (i k) -> p k i", k=9)
                wk = []
                for k in range(9):
                    pt = psum_setup.tile([P, P], BF16, tag="tps")
                    nc.tensor.transpose(pt[:], wq_v[:, k, :], ident[:])
                    wt = const.tile([P, P], BF16, tag=f"w{wi}k{k}")
                    nc.scalar.copy(wt[:], pt[:])
                    wk.append(wt)
                wks.append(wk)
            w1k, w2k = wks

            # image-0 quant starts right behind the weight chains on V, in two
            # splits so conv1(0) chunks 0-2 start after the first pad DMA
            quant1(0, splits=SPLITS)

            invb = []
            for i in ("1", "2"):
                s = setup.tile([P, 1], F32, tag=f"sd{i}")
                nc.scalar.activation(s[:], bnc[f"var{i}"][:], ACTF.Sqrt, bias=eps_b[:])
                r = setup.tile([P, 1], F32, tag=f"rs{i}")
                nc.vector.reciprocal(r[:], s[:])
                inv = const.tile([P, 1], F32, tag=f"inv{i}")
                nc.vector.tensor_tensor(inv[:], bnc[f"gamma{i}"][:], r[:], ALU.mult)
                mi = setup.tile([P, 1], F32, tag=f"mi{i}")
                nc.vector.tensor_tensor(mi[:], bnc[f"mean{i}"][:], inv[:], ALU.mult)
                b = const.tile([P, 1], F32, tag=f"b{i}")
                nc.vector.tensor_tensor(b[:], bnc[f"beta{i}"][:], mi[:], ALU.subtract)
                invb.append((inv, b))
            (inv1, b1), (inv2, b2) = invb

            quant1(1, splits=SPLITS)

        # PSUM pools for the main loop, after psum_setup is released
        psum1_p = ctx.enter_context(tc.tile_pool(name="psum1", bufs=4, space="PSUM"))
        psum2_p = ctx.enter_context(tc.tile_pool(name="psum2", bufs=4, space="PSUM"))

        def conv1_chunks(n, c0, c1):
            if mids[n] is None:
                mids[n] = mid_p.tile([P, HWF], F16, tag="mid", name=f"mid{n}")
            mid = mids[n]

            def evict1(c, ps):
                psv = ps[:].rearrange("p (r w) -> p r w", w=PITCH)[:, :, 1 : 1 + W]
                ov = mid[:, c * 448 : (c + 1) * 448].rearrange("p (r w) -> p r w", w=W)
                nc.scalar.activation(ov, psv, ACTF.Relu, bias=b1[:], scale=inv1[:])

            _emit_conv_chunks(nc, psum1_p, w1k, xq_pads[n % 3][:], c0, c1, evict1)

        def quant2(n):
            u2 = u_p.tile([P, HWF], BF16, tag="u2", name=f"u2_{n}")
            # in-place normalize clobbers mid (dead afterwards)
            _emit_quant_split(nc, small, tmp, mids[n][:], u2[:], NBX, "q2",
                              inplace=True)
            nc.scalar.dma_start(_interior_rows(mq_pads[n % 2], 0, H),
                                u2[:].rearrange("p (h w) -> p h w", w=W))

        def conv2(n):
            if t2s[n] is None:
                t2s[n] = t2_p.tile([P, HWF], F32, tag="t2", name=f"t2_{n}")
            t2 = t2s[n]

            def evict2(c, ps):
                psv = ps[:].rearrange("p (r w) -> p r w", w=PITCH)[:, :, 1 : 1 + W]
                ov = t2[:, c * 448 : (c + 1) * 448].rearrange("p (r w) -> p r w", w=W)
                nc.scalar.activation(ov, psv, ACTF.Identity, bias=b2[:], scale=inv2[:])

            _emit_conv_chunks(nc, psum2_p, w2k, mq_pads[n % 2][:], 0, NCHUNK,
                              evict2)

        def final(n):
            t2 = t2s[n]
            nc.vector.tensor_tensor(t2[:], t2[:], xraws[n][:], ALU.add)
            nc.scalar.activation(t2[:], t2[:], ACTF.Relu, bias=zero_b[:])
            nc.scalar.dma_start(out_d[n].rearrange("c h w -> c (h w)"), t2[:])

        # ---- software pipeline (quant1(0)/quant1(1) emitted during setup) ----
        conv1_chunks(0, 0, NCHUNK)
        # Deep pipeline: conv2 runs one iteration behind quant2, so the
        # quant2(n) chain has a full iteration of slack before conv2(n) needs
        # its pads.  Finals run two iterations behind and lead the V queue
        # (their deps are ancient, so V starts each iteration instantly).
        for n in range(nimg):
            if n >= 1:
                conv2(n - 1)
            if n >= 2:
                final(n - 2)
            if n + 2 < nimg:
                load(n + 2)
            quant2(n)
            if n + 1 < nimg:
                conv1_chunks(n + 1, 0, NCHUNK)
            if n + 2 < nimg:
                quant1(n + 2)
        conv2(nimg - 1)
        final(nimg - 2)
        final(nimg - 1)

    nc.compile()
    return nc


@lru_cache(maxsize=1)
def _get_nc():
    return build_nc(NIMG)


def kernel(x, w1, w2, gamma1, beta1, mean1, var1,
           gamma2, beta2, mean2, var2, _trace=False):
    f = lambda a: np.ascontiguousarray(np.asarray(a, dtype=np.float32))
    x = f(x)
    n_total = x.shape[0]
    assert n_total == N_CORES * NIMG, x.shape
    xs = x.reshape(N_CORES, NIMG, P, H, W)
    rep = {
        "w1": f(w1), "w2": f(w2),
        "gamma1": f(gamma1), "beta1": f(beta1), "mean1": f(mean1), "var1": f(var1),
        "gamma2": f(gamma2), "beta2": f(beta2), "mean2": f(mean2), "var2": f(var2),
    }
    in_maps = [{"x": np.ascontiguousarray(xs[c]), **rep} for c in range(N_CORES)]
    nc = _get_nc()
    res = run_bass_kernel_spmd(nc, in_maps, core_ids=list(range(N_CORES)),
                               trace=_trace)
    out = np.concatenate([res.results[c]["out"] for c in range(N_CORES)], axis=0)
    if _trace:
        kernel.last_result = res
    return out.reshape(n_total, P, H, W)
